# revision 1
# baseline (speedup 1.0000x reference)
"""CNN+SE+LSTM fused Trainium2 kernel.

Data-parallel over batch: B=2048 split across 8 NeuronCores (256 each).
All matmuls run as fp32r (single-pass fp32, ~tf32 precision) on the PE.

Per-core pipeline:
  conv1x1 (c^T layout [u, (b,w)]) -> sigmoid(+bias) -> channel-mean via
  ones-matmul -> SE softmax (tiny matmuls + DRAM-bounce transposes) ->
  broadcast-DMA -> scale + max-over-window on DVE -> 2-layer bidir LSTM
  (single step, h0=c0=0, forget gate dead) -> tanh classifier head.
"""

import numpy as np

import concourse.bass as bass
import concourse.tile as tile
from concourse import bacc, mybir
from concourse.bass_utils import run_bass_kernel_spmd

B, W, D, U, H = 2048, 64, 512, 512, 512
NC = 8
BS = B // NC          # 256 batch rows per core
GB = 8                # batches per group (8 * W = 512 matmul columns)
NG = BS // GB         # 32 groups
DC = D // 128         # 4 contraction chunks
UC = U // 128         # 4 output-channel chunks

dt = mybir.dt
AF = mybir.ActivationFunctionType
ALU = mybir.AluOpType
AX = mybir.AxisListType

_STATE = None


def _build_bass():
    nc = bacc.Bacc("TRN2", target_bir_lowering=False, debug=False, num_devices=NC)

    f32, f32r = dt.float32, dt.float32r

    d_xt = nc.dram_tensor("xt", [D, BS, W], f32r, kind="ExternalInput").ap()
    d_cw = nc.dram_tensor("cw", [128, DC * U], f32r, kind="ExternalInput").ap()
    d_cb = nc.dram_tensor("cb", [128, UC], f32, kind="ExternalInput").ap()
    d_ones = nc.dram_tensor("onescol", [128, 1], f32r, kind="ExternalInput").ap()
    d_ones8 = nc.dram_tensor("ones8", [1, GB], f32r, kind="ExternalInput").ap()
    d_sewt = nc.dram_tensor("sewt", [W, W], f32r, kind="ExternalInput").ap()
    d_seb = nc.dram_tensor("seb", [1, W], f32r, kind="ExternalInput").ap()
    d_w0 = {}
    d_b0 = {}
    d_w1 = {}
    d_b1 = {}
    for s in ("f", "r"):
        d_w0[s] = nc.dram_tensor(f"w0{s}", [128, 4 * 1536], f32r, kind="ExternalInput").ap()
        d_b0[s] = nc.dram_tensor(f"b0{s}", [128, 12], f32, kind="ExternalInput").ap()
        d_w1[s] = nc.dram_tensor(f"w1{s}", [128, 8 * 1536], f32r, kind="ExternalInput").ap()
        d_b1[s] = nc.dram_tensor(f"b1{s}", [128, 12], f32, kind="ExternalInput").ap()
    d_clsw = nc.dram_tensor("clsw", [128, 8], f32r, kind="ExternalInput").ap()
    d_clsb = nc.dram_tensor("clsb", [1, 1], f32, kind="ExternalInput").ap()
    d_out = nc.dram_tensor("out", [1, BS], f32, kind="ExternalOutput").ap()

    with tile.TileContext(nc) as tc:
        with tc.tile_pool(name="wpool", bufs=1) as wpool, \
             tc.tile_pool(name="persist", bufs=1) as persist:
            # static weights, staged up front
            cw_t = wpool.tile([128, DC * U], f32r, name="cw_t")
            nc.sync.dma_start(cw_t[:], d_cw)
            cb_t = wpool.tile([128, UC], f32, name="cb_t")
            nc.sync.dma_start(cb_t[:], d_cb)
            ones_t = wpool.tile([128, 1], f32r, name="ones_t")
            nc.sync.dma_start(ones_t[:], d_ones)
            ones8_t = wpool.tile([1, GB], f32r, name="ones8_t")
            nc.sync.dma_start(ones8_t[:], d_ones8)
            sewt_t = wpool.tile([W, W], f32r, name="sewt_t")
            nc.sync.dma_start(sewt_t[:], d_sewt)
            seb_t = wpool.tile([1, W], f32r, name="seb_t")
            nc.sync.dma_start(seb_t[:], d_seb)
            w0_t, b0_t, w1_t, b1_t = {}, {}, {}, {}
            for s in ("f", "r"):
                w0_t[s] = wpool.tile([128, 4 * 1536], f32r, name=f"w0{s}_t")
                nc.sync.dma_start(w0_t[s][:], d_w0[s])
                b0_t[s] = wpool.tile([128, 12], f32, name=f"b0{s}_t")
                nc.sync.dma_start(b0_t[s][:], d_b0[s])
                w1_t[s] = wpool.tile([128, 8 * 1536], f32r, name=f"w1{s}_t")
                nc.sync.dma_start(w1_t[s][:], d_w1[s])
                b1_t[s] = wpool.tile([128, 12], f32, name=f"b1{s}_t")
                nc.sync.dma_start(b1_t[s][:], d_b1[s])
            clsw_t = wpool.tile([128, 8], f32r, name="clsw_t")
            nc.sync.dma_start(clsw_t[:], d_clsw)
            clsb_t = wpool.tile([1, 1], f32, name="clsb_t")
            nc.sync.dma_start(clsb_t[:], d_clsb)

            # pooled^T accumulators [u_chunk][128, BS], filled per group
            pooledT = []
            for uc in range(UC):
                pt = persist.tile([128, BS], f32r, name=f"pooledT{uc}")
                pooledT.append(pt)

            # ---------------- phase 1: conv + SE + maxpool ----------------
            with tc.tile_pool(name="xp", bufs=8) as xp, \
                 tc.tile_pool(name="sigp", bufs=6) as sigp, \
                 tc.tile_pool(name="scp", bufs=3) as scp, \
                 tc.tile_pool(name="bcp", bufs=2) as bcp, \
                 tc.tile_pool(name="sep", bufs=3) as sep, \
                 tc.tile_pool(name="drp", bufs=3, space="DRAM") as drp, \
                 tc.tile_pool(name="cps", bufs=3, space="PSUM") as cps, \
                 tc.tile_pool(name="usps", bufs=2, space="PSUM") as usps, \
                 tc.tile_pool(name="lgps", bufs=2, space="PSUM") as lgps:
                for g in range(NG):
                    xts = []
                    for dc in range(DC):
                        xt = xp.tile([128, GB, W], f32r, name="xt", tag="x")
                        nc.sync.dma_start(
                            xt[:], d_xt[dc * 128:(dc + 1) * 128, g * GB:(g + 1) * GB, :]
                        )
                        xts.append(xt)

                    us = usps.tile([1, GB * W], f32, name="us", tag="us")
                    sigs = []
                    for uc in range(UC):
                        cp = cps.tile([128, GB * W], f32, name="cp", tag="cp")
                        for dc in range(DC):
                            nc.tensor.matmul(
                                cp[:],
                                cw_t[:, dc * U + uc * 128: dc * U + (uc + 1) * 128],
                                xts[dc][:].rearrange("p b w -> p (b w)"),
                                start=(dc == 0),
                                stop=(dc == DC - 1),
                            )
                        sig = sigp.tile([128, GB * W], f32r, name="sig", tag="sig")
                        nc.scalar.activation(
                            sig[:], cp[:], AF.Sigmoid, bias=cb_t[:, uc:uc + 1], scale=1.0
                        )
                        sigs.append(sig)
                        # channel-sum (mean via 1/U weights): accumulate over uc
                        nc.tensor.matmul(
                            us[:], ones_t[:], sig[:],
                            start=(uc == 0), stop=(uc == UC - 1),
                        )

                    # SE: avg row -> [w, b] via DRAM bounce -> softmax(avg @ se_w.T + se_b)
                    avg_row = sep.tile([1, GB * W], f32r, name="avg_row", tag="avgrow")
                    nc.scalar.copy(avg_row[:], us[:])
                    scr1 = drp.tile([1, GB * W], f32r, name="scr1", tag="scr1")
                    nc.sync.dma_start(scr1[:], avg_row[:])
                    avgT = sep.tile([W, GB], f32r, name="avgT", tag="avgT")
                    nc.sync.dma_start(
                        avgT[:], scr1[:].rearrange("p (b w) -> (p w) b", w=W)
                    )
                    lg = lgps.tile([GB, W], f32, name="lg", tag="lg")
                    nc.tensor.matmul(lg[:], avgT[:], sewt_t[:], start=True, stop=False)
                    nc.tensor.matmul(lg[:], ones8_t[:], seb_t[:], start=False, stop=True)
                    E = sep.tile([GB, W], f32, name="E", tag="E")
                    nc.scalar.activation(E[:], lg[:], AF.Exp)
                    S = sep.tile([GB, 1], f32, name="S", tag="S")
                    nc.vector.reduce_sum(S[:], E[:], axis=AX.X)
                    R = sep.tile([GB, 1], f32, name="R", tag="R")
                    nc.vector.reciprocal(R[:], S[:])
                    seg = sep.tile([GB, W], f32, name="seg", tag="seg")
                    nc.vector.tensor_scalar_mul(seg[:], E[:], R[:, 0:1])
                    scr2 = drp.tile([GB, W], f32, name="scr2", tag="scr2")
                    nc.sync.dma_start(scr2[:], seg[:])
                    sebc = bcp.tile([128, GB * W], f32, name="sebc", tag="sebc")
                    nc.sync.dma_start(
                        sebc[:],
                        scr2[:].rearrange("b w -> (b w)").unsqueeze(0).broadcast_to([128, GB * W]),
                    )

                    for uc in range(UC):
                        scaled = scp.tile([128, GB * W], f32, name="scaled", tag="scaled")
                        nc.vector.tensor_mul(scaled[:], sigs[uc][:].bitcast(dt.float32), sebc[:])
                        nc.vector.tensor_reduce(
                            pooledT[uc][:, g * GB:(g + 1) * GB],
                            scaled[:].rearrange("p (b w) -> p b w", w=W),
                            axis=AX.X,
                            op=ALU.max,
                        )

            # ---------------- phase 2: LSTM + classifier ----------------
            with tc.tile_pool(name="lp", bufs=3) as lp, \
                 tc.tile_pool(name="op", bufs=1) as op, \
                 tc.tile_pool(name="gps", bufs=6, space="PSUM") as gps, \
                 tc.tile_pool(name="clsps", bufs=1, space="PSUM") as clsps:
                GATES = ((0, AF.Sigmoid), (1, AF.Tanh), (2, AF.Sigmoid))  # i, g, o

                def lstm_dir(w_t, b_t, kcs, rhs_tiles, out_tiles, out_tanh):
                    # one direction: 12 useful gate chunks (i,g,o), 4H rows -> 4 q chunks
                    for q in range(4):
                        gate_sb = []
                        for gi, func in GATES:
                            m = gi * 4 + q
                            gp = gps.tile([128, BS], f32, name="gp", tag="gp")
                            for kc in range(kcs):
                                nc.tensor.matmul(
                                    gp[:],
                                    w_t[:, kc * 1536 + m * 128: kc * 1536 + (m + 1) * 128],
                                    rhs_tiles[kc][:],
                                    start=(kc == 0),
                                    stop=(kc == kcs - 1),
                                )
                            gs = lp.tile([128, BS], f32, name="gs", tag=f"gate{gi}")
                            nc.scalar.activation(
                                gs[:], gp[:], func, bias=b_t[:, m:m + 1], scale=1.0
                            )
                            gate_sb.append(gs)
                        si, tg, so = gate_sb
                        cpre = lp.tile([128, BS], f32, name="cpre", tag="cpre")
                        nc.vector.tensor_mul(cpre[:], si[:], tg[:])
                        tcl = lp.tile([128, BS], f32, name="tcl", tag="tcl")
                        nc.scalar.activation(tcl[:], cpre[:], AF.Tanh)
                        if out_tanh:
                            h = lp.tile([128, BS], f32, name="h", tag="h")
                            nc.vector.tensor_mul(h[:], so[:], tcl[:])
                            nc.scalar.activation(out_tiles[q][:], h[:], AF.Tanh)
                        else:
                            nc.vector.tensor_mul(out_tiles[q][:], so[:], tcl[:])

                o0T = [op.tile([128, BS], f32r, name=f"o0T{i}") for i in range(8)]
                o1T = [op.tile([128, BS], f32r, name=f"o1T{i}") for i in range(8)]
                lstm_dir(w0_t["f"], b0_t["f"], 4, pooledT, o0T[0:4], False)
                lstm_dir(w0_t["r"], b0_t["r"], 4, pooledT, o0T[4:8], False)
                lstm_dir(w1_t["f"], b1_t["f"], 8, o0T, o1T[0:4], True)
                lstm_dir(w1_t["r"], b1_t["r"], 8, o0T, o1T[4:8], True)

                clsp = clsps.tile([1, BS], f32, name="clsp")
                for kc in range(8):
                    nc.tensor.matmul(
                        clsp[:], clsw_t[:, kc:kc + 1], o1T[kc][:],
                        start=(kc == 0), stop=(kc == 7),
                    )
                outsb = lp.tile([1, BS], f32, name="outsb", tag="outsb")
                nc.scalar.activation(
                    outsb[:], clsp[:], AF.Tanh, bias=clsb_t[0:1, 0:1], scale=1.0
                )
                nc.sync.dma_start(d_out, outsb[:])

    nc.compile()
    return nc


def _prep_weights(i):
    """Host-side packing of the replicated (non-batch) tensors."""
    def f32(a):
        return np.ascontiguousarray(a, dtype=np.float32)

    out = {}
    out["cw"] = f32(i["conv_w"].T.reshape(DC, 128, U).transpose(1, 0, 2).reshape(128, DC * U))
    out["cb"] = f32(i["conv_b"].reshape(UC, 128).T)
    out["onescol"] = np.full((128, 1), 1.0 / U, np.float32)
    out["ones8"] = np.ones((1, GB), np.float32)
    out["sewt"] = f32(i["se_w"].T)
    out["seb"] = f32(i["se_b"].reshape(1, W))
    igo = np.r_[0:512, 1024:2048]  # drop dead forget gate
    for s, tag in (("f", "l0f"), ("r", "l0r")):
        wT = f32(i[f"w_ih_{tag}"]).T[:, igo]                      # [512, 1536]
        out[f"w0{s}"] = f32(wT.reshape(4, 128, 1536).transpose(1, 0, 2).reshape(128, 4 * 1536))
        bs = (f32(i[f"b_ih_{tag}"]) + f32(i[f"b_hh_{tag}"]))[igo]  # [1536]
        out[f"b0{s}"] = f32(bs.reshape(12, 128).T)
    for s, tag in (("f", "l1f"), ("r", "l1r")):
        wT = f32(i[f"w_ih_{tag}"]).T[:, igo]                      # [1024, 1536]
        out[f"w1{s}"] = f32(wT.reshape(8, 128, 1536).transpose(1, 0, 2).reshape(128, 8 * 1536))
        bs = (f32(i[f"b_ih_{tag}"]) + f32(i[f"b_hh_{tag}"]))[igo]
        out[f"b1{s}"] = f32(bs.reshape(12, 128).T)
    out["clsw"] = f32(i["cls_w"].reshape(2 * H)).reshape(8, 128).T.copy()
    out["clsb"] = f32(i["cls_b"]).reshape(1, 1)
    return out


def _get_nc():
    global _STATE
    if _STATE is None:
        _STATE = _build_bass()
    return _STATE


def make_in_maps(**inputs):
    w = _prep_weights(inputs)
    xt = np.ascontiguousarray(
        np.asarray(inputs["x"], dtype=np.float32).transpose(2, 0, 1)
    )  # [D, B, W]
    maps = []
    for c in range(NC):
        m = dict(w)
        m["xt"] = np.ascontiguousarray(xt[:, c * BS:(c + 1) * BS, :])
        maps.append(m)
    return maps


def kernel(**inputs):
    nc = _get_nc()
    maps = make_in_maps(**inputs)
    res = run_bass_kernel_spmd(nc, maps, core_ids=list(range(NC)))
    out = np.empty((B, 1), np.float32)
    for c in range(NC):
        out[c * BS:(c + 1) * BS, 0] = res.results[c]["out"][0]
    return out


# revision 12
# speedup vs baseline: 20.4898x; 20.4898x over previous
"""CNN+SE+LSTM fused Trainium2 kernel.

Data-parallel over batch: B=2048 split across 8 NeuronCores (256 each).
All matmuls run as fp32r (single-pass fp32, ~tf32 precision) on the PE;
the sigmoid/SE-scale/maxpool tensor path runs in bf16 on the DVE (2x mode).

Per-core pipeline (phase 1, per 8-batch "group", SE batched per block):
  conv1x1 (c^T layout [u, (b,w)]) -> sigmoid(+bias) on ACT (bf16 out) ->
  channel-mean via ones-matmul -> SE softmax (tiny matmuls + DRAM-bounce
  transposes) -> GpSimd cast-DMA broadcast -> bf16 scale + max-over-window
  on DVE -> pooled^T.
Phase 2: 2-layer bidirectional LSTM (single step, h0=c0=0 so the forget gate
is dead and the h@w_hh term vanishes) -> tanh classifier head -> [1, 256].

The final SE blocks are smaller so the tail SE chain doesn't gate the LSTM.
DMA routing: big x loads alternate the two HWDGE rings (SP/ACT); weights and
SE bounce DMAs ride GpSimd SWDGE; w1f prefetches during phase 1.
"""

import numpy as np

import concourse.bass as bass
import concourse.tile as tile
from concourse import bacc, mybir
from concourse.bass_utils import run_bass_kernel_spmd

B, W, D, U, H = 2048, 64, 512, 512, 512
NC = 8
BS = B // NC          # 256 batch rows per core
GB = 8                # batches per group (8 * W = 512 matmul columns)
NG = BS // GB         # 32 groups
BLOCKS = [4, 4, 4, 4, 4, 4, 2, 2, 2, 1, 1]   # SE batching; tapered tail
assert sum(BLOCKS) == NG
DC = D // 128         # 4 contraction chunks
UC = U // 128         # 4 output-channel chunks

dt = mybir.dt
AF = mybir.ActivationFunctionType
ALU = mybir.AluOpType
AX = mybir.AxisListType

_STATE = None


def _build_bass(unroll=1):
    nc = bacc.Bacc("TRN2", target_bir_lowering=False, debug=False,
                   num_devices=NC, num_swdge_queues=4)

    f32, f32r, bf16 = dt.float32, dt.float32r, dt.bfloat16

    d_xt = nc.dram_tensor("xt", [D, BS, W], f32r, kind="ExternalInput").ap()
    d_cw = nc.dram_tensor("cw", [128, DC * U], f32r, kind="ExternalInput").ap()
    d_cb = nc.dram_tensor("cb", [128, UC], f32, kind="ExternalInput").ap()
    d_ones = nc.dram_tensor("onescol", [128, 1], bf16, kind="ExternalInput").ap()
    d_ones32 = nc.dram_tensor("ones32", [1, 4 * GB], f32r, kind="ExternalInput").ap()
    d_sewt = nc.dram_tensor("sewt", [W, W], f32r, kind="ExternalInput").ap()
    d_seb = nc.dram_tensor("seb", [1, W], f32r, kind="ExternalInput").ap()
    d_w0, d_b0, d_w1, d_b1 = {}, {}, {}, {}
    for s in ("f", "r"):
        d_w0[s] = nc.dram_tensor(f"w0{s}", [128, 4 * 1536], f32r, kind="ExternalInput").ap()
        d_b0[s] = nc.dram_tensor(f"b0{s}", [128, 12], f32, kind="ExternalInput").ap()
        d_w1[s] = nc.dram_tensor(f"w1{s}", [128, 8 * 1536], f32r, kind="ExternalInput").ap()
        d_b1[s] = nc.dram_tensor(f"b1{s}", [128, 12], f32, kind="ExternalInput").ap()
    d_clsw = nc.dram_tensor("clsw", [128, 8], f32r, kind="ExternalInput").ap()
    d_clsb = nc.dram_tensor("clsb", [1, 1], f32, kind="ExternalInput").ap()
    d_out = nc.dram_tensor("out", [1, BS], f32, kind="ExternalOutput").ap()

    with tile.TileContext(nc) as tc:
        with tc.tile_pool(name="wpool", bufs=1) as wpool, \
             tc.tile_pool(name="persist", bufs=1) as persist:
            # static weights, staged up front on the SWDGE path
            cw_t = wpool.tile([128, DC * U], f32r, name="cw_t")
            nc.gpsimd.dma_start(cw_t[:], d_cw)
            cb_t = wpool.tile([128, UC], f32, name="cb_t")
            nc.gpsimd.dma_start(cb_t[:], d_cb)
            ones_t = wpool.tile([128, 1], bf16, name="ones_t")
            nc.gpsimd.dma_start(ones_t[:], d_ones)
            ones32_t = wpool.tile([1, 4 * GB], f32r, name="ones32_t")
            nc.gpsimd.dma_start(ones32_t[:], d_ones32)
            sewt_t = wpool.tile([W, W], f32r, name="sewt_t")
            nc.gpsimd.dma_start(sewt_t[:], d_sewt)
            seb_t = wpool.tile([1, W], f32r, name="seb_t")
            nc.gpsimd.dma_start(seb_t[:], d_seb)
            w0_t, b0_t, b1_t = {}, {}, {}
            for s in ("f", "r"):
                w0_t[s] = wpool.tile([128, 4 * 1536], f32r, name=f"w0{s}_t")
                nc.gpsimd.dma_start(w0_t[s][:], d_w0[s])
                b0_t[s] = wpool.tile([128, 12], f32, name=f"b0{s}_t")
                nc.gpsimd.dma_start(b0_t[s][:], d_b0[s])
                b1_t[s] = wpool.tile([128, 12], f32, name=f"b1{s}_t")
                nc.gpsimd.dma_start(b1_t[s][:], d_b1[s])
            clsw_t = wpool.tile([128, 8], f32r, name="clsw_t")
            nc.gpsimd.dma_start(clsw_t[:], d_clsw)
            clsb_t = wpool.tile([1, 1], f32, name="clsb_t")
            nc.gpsimd.dma_start(clsb_t[:], d_clsb)

            # pooled^T accumulator [128, uc, BS], filled per group
            pooledT = persist.tile([128, UC, BS], f32r, name="pooledT")

            # l1 forward weights: prefetched during phase 1 (SWDGE path)
            w1_t = {}
            w1_t["f"] = persist.tile([128, 8 * 1536], f32r, name="w1f_t")
            nc.gpsimd.dma_start(w1_t["f"][:], d_w1["f"])

            for _rep in range(unroll):
                # ---------------- phase 1: conv + SE + maxpool ----------------
                with tc.tile_pool(name="xp", bufs=3) as xp, \
                     tc.tile_pool(name="sigp", bufs=10) as sigp, \
                     tc.tile_pool(name="scp", bufs=2) as scp, \
                     tc.tile_pool(name="bcp", bufs=2) as bcp, \
                     tc.tile_pool(name="sep", bufs=2) as sep, \
                     tc.tile_pool(name="drp", bufs=2, space="DRAM") as drp, \
                     tc.tile_pool(name="cps", bufs=5, space="PSUM") as cps, \
                     tc.tile_pool(name="usps", bufs=2, space="PSUM") as usps, \
                     tc.tile_pool(name="lgps", bufs=1, space="PSUM") as lgps:
                    g0 = 0
                    for nblk in BLOCKS:
                        gs = list(range(g0, g0 + nblk))
                        g0 += nblk
                        scr1 = drp.tile([4, GB * W], f32r, name="scr1", tag="scr1")
                        sig_blk = []
                        for gi, g in enumerate(gs):
                            xt = xp.tile([128, DC, GB * W], f32r, name="xt", tag="x")
                            # one 512 KiB DMA per group, alternating HWDGE rings
                            dma_eng = nc.sync if g % 2 == 0 else nc.scalar
                            src = d_xt[:, g * GB:(g + 1) * GB, :].rearrange(
                                "(dc p) b w -> p dc (b w)", p=128
                            )
                            dma_eng.dma_start(xt[:], src)

                            us = usps.tile([1, GB * W], f32, name="us", tag="us")
                            sigg = sigp.tile([128, UC, GB * W], bf16, name="sigg", tag="sig")
                            sig_blk.append(sigg)
                            for uc in range(UC):
                                cp = cps.tile([128, GB * W], f32, name="cp", tag="cp")
                                for dc in range(DC):
                                    nc.tensor.matmul(
                                        cp[:],
                                        cw_t[:, dc * U + uc * 128: dc * U + (uc + 1) * 128],
                                        xt[:, dc, :],
                                        start=(dc == 0),
                                        stop=(dc == DC - 1),
                                    )
                                nc.scalar.activation(
                                    sigg[:, uc, :], cp[:], AF.Sigmoid,
                                    bias=cb_t[:, uc:uc + 1], scale=1.0,
                                )
                                nc.tensor.matmul(
                                    us[:], ones_t[:], sigg[:, uc, :],
                                    start=(uc == 0), stop=(uc == UC - 1),
                                )
                            # avg row -> DRAM scratch (DVE copy keeps ACT on Sigmoid)
                            avg_row = sep.tile([1, GB * W], f32r, name="avg_row", tag="avgrow")
                            nc.vector.tensor_copy(avg_row[:], us[:])
                            nc.sync.dma_start(scr1[gi:gi + 1, :], avg_row[:])

                        # SE for the whole block: avgT [w, (gi b)]
                        nb = len(gs) * GB
                        avgT = sep.tile([W, 4 * GB], f32r, name="avgT", tag="avgT")
                        nc.sync.dma_start(
                            avgT[:, 0:nb],
                            scr1[0:len(gs), :].rearrange("g (b w) -> (w) g b", w=W),
                        )
                        lg = lgps.tile([4 * GB, W], f32, name="lg", tag="lg")
                        nc.tensor.matmul(lg[0:nb, :], avgT[:, 0:nb], sewt_t[:],
                                         start=True, stop=False)
                        nc.tensor.matmul(lg[0:nb, :], ones32_t[:, 0:nb], seb_t[:],
                                         start=False, stop=True)
                        E = sep.tile([4 * GB, W], f32, name="E", tag="E")
                        nc.scalar.activation(E[0:nb, :], lg[0:nb, :], AF.Exp)
                        S = sep.tile([4 * GB, 1], f32, name="S", tag="S")
                        nc.vector.reduce_sum(S[0:nb, :], E[0:nb, :], axis=AX.X)
                        R = sep.tile([4 * GB, 1], f32, name="R", tag="R")
                        nc.vector.reciprocal(R[0:nb, :], S[0:nb, :])
                        seg = sep.tile([4 * GB, W], f32r, name="seg", tag="seg")
                        nc.vector.tensor_scalar_mul(seg[0:nb, :], E[0:nb, :], R[0:nb, 0:1])
                        scr2 = drp.tile([4 * GB, W], f32r, name="scr2", tag="scr2")
                        nc.scalar.dma_start(scr2[0:nb, :], seg[0:nb, :])
                        # broadcast to all partitions with f32r->bf16 cast (SWDGE)
                        sebc = bcp.tile([128, 4 * GB * W], bf16, name="sebc", tag="sebc")
                        nc.gpsimd.dma_start(
                            sebc[:, 0:nb * W],
                            scr2[0:nb, :].bitcast(f32)
                            .rearrange("b w -> (b w)").unsqueeze(0)
                            .broadcast_to([128, nb * W]),
                        )
                        for gi, g in enumerate(gs):
                            scaled = scp.tile([128, UC, GB * W], bf16, name="scaled", tag="scaled")
                            nc.vector.tensor_mul(
                                scaled[:],
                                sig_blk[gi][:],
                                sebc[:, gi * GB * W:(gi + 1) * GB * W]
                                .unsqueeze(1).broadcast_to([128, UC, GB * W]),
                            )
                            pbf = scp.tile([128, UC * GB], bf16, name="pbf", tag="pbf")
                            nc.vector.tensor_reduce(
                                pbf[:],
                                scaled[:].rearrange("p u (b w) -> p (u b) w", w=W),
                                axis=AX.X,
                                op=ALU.max,
                            )
                            nc.vector.tensor_copy(
                                pooledT[:, :, g * GB:(g + 1) * GB],
                                pbf[:].rearrange("p (u b) -> p u b", u=UC),
                            )

                # ---------------- phase 2: LSTM + classifier ----------------
                with tc.tile_pool(name="w1rp", bufs=1) as w1rp, \
                     tc.tile_pool(name="lp", bufs=2) as lp, \
                     tc.tile_pool(name="op", bufs=1) as op, \
                     tc.tile_pool(name="gps", bufs=6, space="PSUM") as gps, \
                     tc.tile_pool(name="clsps", bufs=1, space="PSUM") as clsps:
                    w1_t["r"] = w1rp.tile([128, 8 * 1536], f32r, name="w1r_t")
                    nc.gpsimd.dma_start(w1_t["r"][:], d_w1["r"])

                    def lstm_dir(w_t, b_t, kcs, rhs_tiles, out_tiles, out_tanh):
                        # i/o gates first (Sigmoid run), then g + tanh(c) (Tanh run)
                        gate_sb = {}
                        for gi, func in ((0, AF.Sigmoid), (2, AF.Sigmoid), (1, AF.Tanh)):
                            for q in range(4):
                                m = gi * 4 + q
                                gp = gps.tile([128, BS], f32, name="gp", tag="gp")
                                for kc in range(kcs):
                                    nc.tensor.matmul(
                                        gp[:],
                                        w_t[:, kc * 1536 + m * 128: kc * 1536 + (m + 1) * 128],
                                        rhs_tiles[kc],
                                        start=(kc == 0),
                                        stop=(kc == kcs - 1),
                                    )
                                gs_ = lp.tile([128, BS], f32, name="gs", tag=f"gate{gi}q{q}")
                                nc.scalar.activation(
                                    gs_[:], gp[:], func, bias=b_t[:, m:m + 1], scale=1.0
                                )
                                gate_sb[(gi, q)] = gs_
                        for q in range(4):
                            cpre = lp.tile([128, BS], f32, name="cpre", tag="cpre")
                            nc.vector.tensor_mul(cpre[:], gate_sb[(0, q)][:], gate_sb[(1, q)][:])
                            tcl = lp.tile([128, BS], f32, name="tcl", tag="tcl")
                            nc.scalar.activation(tcl[:], cpre[:], AF.Tanh)
                            if out_tanh:
                                h = lp.tile([128, BS], f32, name="h", tag="h")
                                nc.vector.tensor_mul(h[:], gate_sb[(2, q)][:], tcl[:])
                                nc.scalar.activation(out_tiles[q], h[:], AF.Tanh)
                            else:
                                nc.vector.tensor_mul(out_tiles[q], gate_sb[(2, q)][:], tcl[:])

                    o0T = [op.tile([128, BS], f32r, name=f"o0T{i}")[:] for i in range(8)]
                    o1T = [op.tile([128, BS], f32r, name=f"o1T{i}")[:] for i in range(8)]
                    pooled_rhs = [pooledT[:, kc, :] for kc in range(UC)]
                    lstm_dir(w0_t["f"], b0_t["f"], 4, pooled_rhs, o0T[0:4], False)
                    lstm_dir(w0_t["r"], b0_t["r"], 4, pooled_rhs, o0T[4:8], False)
                    lstm_dir(w1_t["f"], b1_t["f"], 8, o0T, o1T[0:4], True)
                    lstm_dir(w1_t["r"], b1_t["r"], 8, o0T, o1T[4:8], True)

                    clsp = clsps.tile([1, BS], f32, name="clsp")
                    for kc in range(8):
                        nc.tensor.matmul(
                            clsp[:], clsw_t[:, kc:kc + 1], o1T[kc],
                            start=(kc == 0), stop=(kc == 7),
                        )
                    outsb = lp.tile([1, BS], f32, name="outsb", tag="outsb")
                    nc.scalar.activation(
                        outsb[:], clsp[:], AF.Tanh, bias=clsb_t[0:1, 0:1], scale=1.0
                    )
                    nc.sync.dma_start(d_out, outsb[:])

    nc.compile()
    return nc


def _prep_weights(i):
    """Host-side packing of the replicated (non-batch) tensors."""
    import ml_dtypes

    def f32(a):
        return np.ascontiguousarray(a, dtype=np.float32)

    out = {}
    out["cw"] = f32(i["conv_w"].T.reshape(DC, 128, U).transpose(1, 0, 2).reshape(128, DC * U))
    out["cb"] = f32(i["conv_b"].reshape(UC, 128).T)
    out["onescol"] = np.full((128, 1), 1.0 / U, ml_dtypes.bfloat16)
    out["ones32"] = np.ones((1, 4 * GB), np.float32)
    out["sewt"] = f32(i["se_w"].T)
    out["seb"] = f32(i["se_b"].reshape(1, W))
    igo = np.r_[0:512, 1024:2048]  # drop dead forget gate
    for s, tag in (("f", "l0f"), ("r", "l0r")):
        wT = f32(i[f"w_ih_{tag}"]).T[:, igo]                      # [512, 1536]
        out[f"w0{s}"] = f32(wT.reshape(4, 128, 1536).transpose(1, 0, 2).reshape(128, 4 * 1536))
        bs = (f32(i[f"b_ih_{tag}"]) + f32(i[f"b_hh_{tag}"]))[igo]  # [1536]
        out[f"b0{s}"] = f32(bs.reshape(12, 128).T)
    for s, tag in (("f", "l1f"), ("r", "l1r")):
        wT = f32(i[f"w_ih_{tag}"]).T[:, igo]                      # [1024, 1536]
        out[f"w1{s}"] = f32(wT.reshape(8, 128, 1536).transpose(1, 0, 2).reshape(128, 8 * 1536))
        bs = (f32(i[f"b_ih_{tag}"]) + f32(i[f"b_hh_{tag}"]))[igo]
        out[f"b1{s}"] = f32(bs.reshape(12, 128).T)
    out["clsw"] = f32(i["cls_w"].reshape(2 * H)).reshape(8, 128).T.copy()
    out["clsb"] = f32(i["cls_b"]).reshape(1, 1)
    return out


def _get_nc():
    global _STATE
    if _STATE is None:
        _STATE = _build_bass()
    return _STATE


def make_in_maps(**inputs):
    w = _prep_weights(inputs)
    xt = np.ascontiguousarray(
        np.asarray(inputs["x"], dtype=np.float32).transpose(2, 0, 1)
    )  # [D, B, W]
    maps = []
    for c in range(NC):
        m = dict(w)
        m["xt"] = np.ascontiguousarray(xt[:, c * BS:(c + 1) * BS, :])
        maps.append(m)
    return maps


def kernel(**inputs):
    nc = _get_nc()
    maps = make_in_maps(**inputs)
    res = run_bass_kernel_spmd(nc, maps, core_ids=list(range(NC)))
    out = np.empty((B, 1), np.float32)
    for c in range(NC):
        out[c * BS:(c + 1) * BS, 0] = res.results[c]["out"][0]
    return out


# revision 15
# speedup vs baseline: 20.6145x; 1.0061x over previous
"""CNN+SE+LSTM fused Trainium2 kernel.

Data-parallel over batch: B=2048 split across 8 NeuronCores (256 each).
All matmuls run as fp32r (single-pass fp32, ~tf32 precision) on the PE;
the sigmoid/SE-scale/maxpool tensor path runs in bf16 on the DVE (2x mode).

Per-core pipeline (phase 1, per 8-batch "group", SE batched per block):
  conv1x1 (c^T layout [u, (b,w)]) -> sigmoid(+bias) on ACT (bf16 out) ->
  channel-mean via ones-matmul -> SE softmax (tiny matmuls + DRAM-bounce
  transposes) -> GpSimd cast-DMA broadcast -> bf16 scale + max-over-window
  on DVE -> pooled^T.
Phase 2: 2-layer bidirectional LSTM (single step, h0=c0=0 so the forget gate
is dead and the h@w_hh term vanishes) -> tanh classifier head -> [1, 256].

The final SE blocks are smaller so the tail SE chain doesn't gate the LSTM.
DMA routing: big x loads alternate the two HWDGE rings (SP/ACT); weights and
SE bounce DMAs ride GpSimd SWDGE; w1f prefetches during phase 1.
"""

import numpy as np

import concourse.bass as bass
import concourse.tile as tile
from concourse import bacc, mybir
from concourse.bass_utils import run_bass_kernel_spmd

B, W, D, U, H = 2048, 64, 512, 512, 512
NC = 8
BS = B // NC          # 256 batch rows per core
GB = 8                # batches per group (8 * W = 512 matmul columns)
NG = BS // GB         # 32 groups
BLOCKS = [4, 4, 4, 4, 4, 4, 2, 2, 2, 1, 1]   # SE batching; tapered tail
assert sum(BLOCKS) == NG
DC = D // 128         # 4 contraction chunks
UC = U // 128         # 4 output-channel chunks

dt = mybir.dt
AF = mybir.ActivationFunctionType
ALU = mybir.AluOpType
AX = mybir.AxisListType

_STATE = None


def _build_bass(unroll=1):
    nc = bacc.Bacc("TRN2", target_bir_lowering=False, debug=False,
                   num_devices=NC, num_swdge_queues=4)

    f32, f32r, bf16 = dt.float32, dt.float32r, dt.bfloat16

    d_xt = nc.dram_tensor("xt", [D, BS, W], f32r, kind="ExternalInput").ap()
    d_cw = nc.dram_tensor("cw", [128, DC * U], f32r, kind="ExternalInput").ap()
    d_cb = nc.dram_tensor("cb", [128, UC], f32, kind="ExternalInput").ap()
    d_ones = nc.dram_tensor("onescol", [128, 1], bf16, kind="ExternalInput").ap()
    d_ones32 = nc.dram_tensor("ones32", [1, 4 * GB], f32r, kind="ExternalInput").ap()
    d_sewt = nc.dram_tensor("sewt", [W, W], f32r, kind="ExternalInput").ap()
    d_seb = nc.dram_tensor("seb", [1, W], f32r, kind="ExternalInput").ap()
    d_w0, d_b0, d_w1, d_b1 = {}, {}, {}, {}
    for s in ("f", "r"):
        d_w0[s] = nc.dram_tensor(f"w0{s}", [128, 4 * 1536], f32r, kind="ExternalInput").ap()
        d_b0[s] = nc.dram_tensor(f"b0{s}", [128, 12], f32, kind="ExternalInput").ap()
        d_w1[s] = nc.dram_tensor(f"w1{s}", [128, 8 * 1536], f32r, kind="ExternalInput").ap()
        d_b1[s] = nc.dram_tensor(f"b1{s}", [128, 12], f32, kind="ExternalInput").ap()
    d_clsw = nc.dram_tensor("clsw", [128, 8], f32r, kind="ExternalInput").ap()
    d_clsb = nc.dram_tensor("clsb", [1, 1], f32, kind="ExternalInput").ap()
    d_out = nc.dram_tensor("out", [1, BS], f32, kind="ExternalOutput").ap()

    with tile.TileContext(nc) as tc:
        with tc.tile_pool(name="wpool", bufs=1) as wpool, \
             tc.tile_pool(name="persist", bufs=1) as persist:
            # static weights, staged up front on the SWDGE path
            cw_t = wpool.tile([128, DC * U], f32r, name="cw_t")
            nc.gpsimd.dma_start(cw_t[:], d_cw)
            cb_t = wpool.tile([128, UC], f32, name="cb_t")
            nc.gpsimd.dma_start(cb_t[:], d_cb)
            ones_t = wpool.tile([128, 1], bf16, name="ones_t")
            nc.gpsimd.dma_start(ones_t[:], d_ones)
            ones32_t = wpool.tile([1, 4 * GB], f32r, name="ones32_t")
            nc.gpsimd.dma_start(ones32_t[:], d_ones32)
            sewt_t = wpool.tile([W, W], f32r, name="sewt_t")
            nc.gpsimd.dma_start(sewt_t[:], d_sewt)
            seb_t = wpool.tile([1, W], f32r, name="seb_t")
            nc.gpsimd.dma_start(seb_t[:], d_seb)
            w0_t, b0_t, b1_t = {}, {}, {}
            for s in ("f", "r"):
                w0_t[s] = wpool.tile([128, 4 * 1536], f32r, name=f"w0{s}_t")
                nc.gpsimd.dma_start(w0_t[s][:], d_w0[s])
                b0_t[s] = wpool.tile([128, 12], f32, name=f"b0{s}_t")
                nc.gpsimd.dma_start(b0_t[s][:], d_b0[s])
                b1_t[s] = wpool.tile([128, 12], f32, name=f"b1{s}_t")
                nc.gpsimd.dma_start(b1_t[s][:], d_b1[s])
            clsw_t = wpool.tile([128, 8], f32r, name="clsw_t")
            nc.gpsimd.dma_start(clsw_t[:], d_clsw)
            clsb_t = wpool.tile([1, 1], f32, name="clsb_t")
            nc.gpsimd.dma_start(clsb_t[:], d_clsb)

            # pooled^T accumulator [128, uc, BS], filled per group
            pooledT = persist.tile([128, UC, BS], f32r, name="pooledT")

            # l1 forward weights: prefetched during phase 1 (SWDGE path)
            w1_t = {}
            w1_t["f"] = persist.tile([128, 8 * 1536], f32r, name="w1f_t")
            nc.gpsimd.dma_start(w1_t["f"][:], d_w1["f"])

            for _rep in range(unroll):
                # ---------------- phase 1: conv + SE + maxpool ----------------
                with tc.tile_pool(name="xp", bufs=3) as xp, \
                     tc.tile_pool(name="sigp", bufs=10) as sigp, \
                     tc.tile_pool(name="scp", bufs=2) as scp, \
                     tc.tile_pool(name="bcp", bufs=2) as bcp, \
                     tc.tile_pool(name="sep", bufs=2) as sep, \
                     tc.tile_pool(name="drp", bufs=2, space="DRAM") as drp, \
                     tc.tile_pool(name="cps", bufs=5, space="PSUM") as cps, \
                     tc.tile_pool(name="usps", bufs=2, space="PSUM") as usps, \
                     tc.tile_pool(name="lgps", bufs=1, space="PSUM") as lgps:
                    g0 = 0
                    for nblk in BLOCKS:
                        gs = list(range(g0, g0 + nblk))
                        g0 += nblk
                        scr1 = drp.tile([4, GB * W], f32r, name="scr1", tag="scr1")
                        sig_blk = []
                        for gi, g in enumerate(gs):
                            xt = xp.tile([128, DC, GB * W], f32r, name="xt", tag="x")
                            # one 512 KiB DMA per group, alternating HWDGE rings
                            dma_eng = nc.sync if g % 2 == 0 else nc.scalar
                            src = d_xt[:, g * GB:(g + 1) * GB, :].rearrange(
                                "(dc p) b w -> p dc (b w)", p=128
                            )
                            dma_eng.dma_start(xt[:], src)

                            us = usps.tile([1, GB * W], f32, name="us", tag="us")
                            sigg = sigp.tile([128, UC, GB * W], bf16, name="sigg", tag="sig")
                            sig_blk.append(sigg)
                            for uc in range(UC):
                                cp = cps.tile([128, GB * W], f32, name="cp", tag="cp")
                                for dc in range(DC):
                                    nc.tensor.matmul(
                                        cp[:],
                                        cw_t[:, dc * U + uc * 128: dc * U + (uc + 1) * 128],
                                        xt[:, dc, :],
                                        start=(dc == 0),
                                        stop=(dc == DC - 1),
                                    )
                                nc.scalar.activation(
                                    sigg[:, uc, :], cp[:], AF.Sigmoid,
                                    bias=cb_t[:, uc:uc + 1], scale=1.0,
                                )
                                nc.tensor.matmul(
                                    us[:], ones_t[:], sigg[:, uc, :],
                                    start=(uc == 0), stop=(uc == UC - 1),
                                )
                            # avg row -> DRAM scratch (DVE copy keeps ACT on Sigmoid)
                            avg_row = sep.tile([1, GB * W], f32r, name="avg_row", tag="avgrow")
                            nc.vector.tensor_copy(avg_row[:], us[:])
                            nc.sync.dma_start(scr1[gi:gi + 1, :], avg_row[:])

                        # SE for the whole block: avgT [w, (gi b)]
                        nb = len(gs) * GB
                        avgT = sep.tile([W, 4 * GB], f32r, name="avgT", tag="avgT")
                        nc.sync.dma_start(
                            avgT[:, 0:nb],
                            scr1[0:len(gs), :].rearrange("g (b w) -> (w) g b", w=W),
                        )
                        lg = lgps.tile([4 * GB, W], f32, name="lg", tag="lg")
                        nc.tensor.matmul(lg[0:nb, :], avgT[:, 0:nb], sewt_t[:],
                                         start=True, stop=False)
                        nc.tensor.matmul(lg[0:nb, :], ones32_t[:, 0:nb], seb_t[:],
                                         start=False, stop=True)
                        E = sep.tile([4 * GB, W], f32, name="E", tag="E")
                        nc.scalar.activation(E[0:nb, :], lg[0:nb, :], AF.Exp)
                        S = sep.tile([4 * GB, 1], f32, name="S", tag="S")
                        nc.vector.reduce_sum(S[0:nb, :], E[0:nb, :], axis=AX.X)
                        R = sep.tile([4 * GB, 1], f32, name="R", tag="R")
                        nc.vector.reciprocal(R[0:nb, :], S[0:nb, :])
                        seg = sep.tile([4 * GB, W], f32r, name="seg", tag="seg")
                        nc.vector.tensor_scalar_mul(seg[0:nb, :], E[0:nb, :], R[0:nb, 0:1])
                        scr2 = drp.tile([4 * GB, W], f32r, name="scr2", tag="scr2")
                        nc.scalar.dma_start(scr2[0:nb, :], seg[0:nb, :])
                        # broadcast to all partitions with f32r->bf16 cast (SWDGE)
                        sebc = bcp.tile([128, 4 * GB * W], bf16, name="sebc", tag="sebc")
                        nc.gpsimd.dma_start(
                            sebc[:, 0:nb * W],
                            scr2[0:nb, :].bitcast(f32)
                            .rearrange("b w -> (b w)").unsqueeze(0)
                            .broadcast_to([128, nb * W]),
                        )
                        for gi, g in enumerate(gs):
                            scaled = scp.tile([128, UC, GB * W], bf16, name="scaled", tag="scaled")
                            nc.vector.tensor_mul(
                                scaled[:],
                                sig_blk[gi][:],
                                sebc[:, gi * GB * W:(gi + 1) * GB * W]
                                .unsqueeze(1).broadcast_to([128, UC, GB * W]),
                            )
                            pbf = scp.tile([128, UC * GB], bf16, name="pbf", tag="pbf")
                            nc.vector.tensor_reduce(
                                pbf[:],
                                scaled[:].rearrange("p u (b w) -> p (u b) w", w=W),
                                axis=AX.X,
                                op=ALU.max,
                            )
                            nc.vector.tensor_copy(
                                pooledT[:, :, g * GB:(g + 1) * GB],
                                pbf[:].rearrange("p (u b) -> p u b", u=UC),
                            )

                # ---------------- phase 2: LSTM + classifier ----------------
                with tc.tile_pool(name="w1rp", bufs=1) as w1rp, \
                     tc.tile_pool(name="lp", bufs=2) as lp, \
                     tc.tile_pool(name="op", bufs=1) as op, \
                     tc.tile_pool(name="gps", bufs=6, space="PSUM") as gps, \
                     tc.tile_pool(name="clsps", bufs=1, space="PSUM") as clsps:
                    w1_t["r"] = w1rp.tile([128, 8 * 1536], f32r, name="w1r_t")
                    nc.gpsimd.dma_start(w1_t["r"][:], d_w1["r"])

                    def lstm_dir(w_t, b_t, kcs, rhs_tiles, out_tiles, out_tanh):
                        # i/o gates first (Sigmoid run), then g + tanh(c) (Tanh run)
                        gate_sb = {}
                        for gi, func in ((0, AF.Sigmoid), (2, AF.Sigmoid), (1, AF.Tanh)):
                            for q in range(4):
                                m = gi * 4 + q
                                gp = gps.tile([128, BS], f32, name="gp", tag="gp")
                                for kc in range(kcs):
                                    nc.tensor.matmul(
                                        gp[:],
                                        w_t[:, kc * 1536 + m * 128: kc * 1536 + (m + 1) * 128],
                                        rhs_tiles[kc],
                                        start=(kc == 0),
                                        stop=(kc == kcs - 1),
                                    )
                                gs_ = lp.tile([128, BS], f32, name="gs", tag=f"gate{gi}q{q}")
                                nc.scalar.activation(
                                    gs_[:], gp[:], func, bias=b_t[:, m:m + 1], scale=1.0
                                )
                                gate_sb[(gi, q)] = gs_
                        for q in range(4):
                            cpre = lp.tile([128, BS], f32, name="cpre", tag="cpre")
                            nc.vector.tensor_mul(cpre[:], gate_sb[(0, q)][:], gate_sb[(1, q)][:])
                            tcl = lp.tile([128, BS], f32, name="tcl", tag="tcl")
                            nc.scalar.activation(tcl[:], cpre[:], AF.Tanh)
                            if out_tanh:
                                h = lp.tile([128, BS], f32, name="h", tag="h")
                                nc.vector.tensor_mul(h[:], gate_sb[(2, q)][:], tcl[:])
                                nc.scalar.activation(out_tiles[q], h[:], AF.Tanh)
                            else:
                                nc.vector.tensor_mul(out_tiles[q], gate_sb[(2, q)][:], tcl[:])

                    o0T = [op.tile([128, BS], f32r, name=f"o0T{i}")[:] for i in range(8)]
                    o1T = [op.tile([128, BS], f32r, name=f"o1T{i}")[:] for i in range(8)]
                    pooled_rhs = [pooledT[:, kc, :] for kc in range(UC)]
                    lstm_dir(w0_t["f"], b0_t["f"], 4, pooled_rhs, o0T[0:4], False)
                    lstm_dir(w0_t["r"], b0_t["r"], 4, pooled_rhs, o0T[4:8], False)
                    lstm_dir(w1_t["f"], b1_t["f"], 8, o0T, o1T[0:4], True)
                    lstm_dir(w1_t["r"], b1_t["r"], 8, o0T, o1T[4:8], True)

                    clsp = clsps.tile([1, BS], f32, name="clsp")
                    for kc in range(8):
                        nc.tensor.matmul(
                            clsp[:], clsw_t[:, kc:kc + 1], o1T[kc],
                            start=(kc == 0), stop=(kc == 7),
                        )
                    outsb = lp.tile([1, BS], f32, name="outsb", tag="outsb")
                    nc.scalar.activation(
                        outsb[:], clsp[:], AF.Tanh, bias=clsb_t[0:1, 0:1], scale=1.0
                    )
                    nc.sync.dma_start(d_out, outsb[:])

    nc.compile()
    return nc


def _prep_weights(i):
    """Host-side packing of the replicated (non-batch) tensors."""
    import ml_dtypes

    def f32(a):
        return np.ascontiguousarray(a, dtype=np.float32)

    out = {}
    out["cw"] = f32(i["conv_w"].T.reshape(DC, 128, U).transpose(1, 0, 2).reshape(128, DC * U))
    out["cb"] = f32(i["conv_b"].reshape(UC, 128).T)
    out["onescol"] = np.full((128, 1), 1.0 / U, ml_dtypes.bfloat16)
    out["ones32"] = np.ones((1, 4 * GB), np.float32)
    out["sewt"] = f32(i["se_w"].T)
    out["seb"] = f32(i["se_b"].reshape(1, W))
    igo = np.r_[0:512, 1024:2048]  # drop dead forget gate
    for s, tag in (("f", "l0f"), ("r", "l0r")):
        wT = f32(i[f"w_ih_{tag}"]).T[:, igo]                      # [512, 1536]
        out[f"w0{s}"] = f32(wT.reshape(4, 128, 1536).transpose(1, 0, 2).reshape(128, 4 * 1536))
        bs = (f32(i[f"b_ih_{tag}"]) + f32(i[f"b_hh_{tag}"]))[igo]  # [1536]
        out[f"b0{s}"] = f32(bs.reshape(12, 128).T)
    for s, tag in (("f", "l1f"), ("r", "l1r")):
        wT = f32(i[f"w_ih_{tag}"]).T[:, igo]                      # [1024, 1536]
        out[f"w1{s}"] = f32(wT.reshape(8, 128, 1536).transpose(1, 0, 2).reshape(128, 8 * 1536))
        bs = (f32(i[f"b_ih_{tag}"]) + f32(i[f"b_hh_{tag}"]))[igo]
        out[f"b1{s}"] = f32(bs.reshape(12, 128).T)
    out["clsw"] = f32(i["cls_w"].reshape(2 * H)).reshape(8, 128).T.copy()
    out["clsb"] = f32(i["cls_b"]).reshape(1, 1)
    return out


def _get_nc():
    global _STATE
    if _STATE is None:
        _STATE = _build_bass()
    return _STATE


def make_in_maps(**inputs):
    w = _prep_weights(inputs)
    xt = np.ascontiguousarray(
        np.asarray(inputs["x"], dtype=np.float32).transpose(2, 0, 1)
    )  # [D, B, W]
    maps = []
    for c in range(NC):
        m = dict(w)
        m["xt"] = np.ascontiguousarray(xt[:, c * BS:(c + 1) * BS, :])
        maps.append(m)
    return maps


def kernel(**inputs):
    nc = _get_nc()
    maps = make_in_maps(**inputs)
    res = run_bass_kernel_spmd(nc, maps, core_ids=list(range(NC)))
    out = np.empty((B, 1), np.float32)
    for c in range(NC):
        out[c * BS:(c + 1) * BS, 0] = res.results[c]["out"][0]
    return out


# revision 20
# speedup vs baseline: 21.3172x; 1.0341x over previous
"""CNN+SE+LSTM fused Trainium2 kernel.

Data-parallel over batch: B=2048 split across 8 NeuronCores (256 each).
All matmuls run as fp32r (single-pass fp32, ~tf32 precision) on the PE;
the sigmoid/SE-scale/maxpool tensor path runs in bf16 on the DVE (2x mode).

Per-core pipeline (phase 1, per 8-batch "group", SE batched per block):
  conv1x1 (c^T layout [u, (b,w)]) -> sigmoid(+bias) on ACT (bf16 out) ->
  channel-mean via ones-matmul -> SE softmax (tiny matmuls + DRAM-bounce
  transposes) -> GpSimd cast-DMA broadcast -> bf16 scale + max-over-window
  on DVE -> pooled^T.
Phase 2: 2-layer bidirectional LSTM (single step, h0=c0=0 so the forget gate
is dead and the h@w_hh term vanishes) -> tanh classifier head -> [1, 256].

The final SE blocks are smaller so the tail SE chain doesn't gate the LSTM.
DMA routing: big x loads alternate the two HWDGE rings (SP/ACT); weights and
SE bounce DMAs ride GpSimd SWDGE; w1f prefetches during phase 1.
"""

import numpy as np

import concourse.bass as bass
import concourse.tile as tile
from concourse import bacc, mybir
from concourse.bass_utils import run_bass_kernel_spmd

B, W, D, U, H = 2048, 64, 512, 512, 512
NC = 8
BS = B // NC          # 256 batch rows per core
GB = 8                # batches per group (8 * W = 512 matmul columns)
NG = BS // GB         # 32 groups
BLOCKS = [4, 4, 4, 4, 4, 4, 2, 2, 2, 1, 1]   # SE batching; tapered tail
assert sum(BLOCKS) == NG
DC = D // 128         # 4 contraction chunks
UC = U // 128         # 4 output-channel chunks

dt = mybir.dt
AF = mybir.ActivationFunctionType
ALU = mybir.AluOpType
AX = mybir.AxisListType

_STATE = None


def _build_bass(unroll=1):
    nc = bacc.Bacc("TRN2", target_bir_lowering=False, debug=False,
                   num_devices=NC, num_swdge_queues=4)

    f32, f32r, bf16 = dt.float32, dt.float32r, dt.bfloat16

    d_xt = nc.dram_tensor("xt", [D, BS, W], f32r, kind="ExternalInput").ap()
    d_cw = nc.dram_tensor("cw", [128, DC * U], f32r, kind="ExternalInput").ap()
    d_cb = nc.dram_tensor("cb", [128, UC], f32, kind="ExternalInput").ap()
    d_ones = nc.dram_tensor("onescol", [128, 1], bf16, kind="ExternalInput").ap()
    d_ones32 = nc.dram_tensor("ones32", [1, 4 * GB], f32r, kind="ExternalInput").ap()
    d_sewt = nc.dram_tensor("sewt", [W, W], f32r, kind="ExternalInput").ap()
    d_seb = nc.dram_tensor("seb", [1, W], f32r, kind="ExternalInput").ap()
    d_w0, d_b0, d_w1, d_b1 = {}, {}, {}, {}
    for s in ("f", "r"):
        d_w0[s] = nc.dram_tensor(f"w0{s}", [128, 4 * 1536], f32r, kind="ExternalInput").ap()
        d_b0[s] = nc.dram_tensor(f"b0{s}", [128, 12], f32, kind="ExternalInput").ap()
        d_w1[s] = nc.dram_tensor(f"w1{s}", [128, 8 * 1536], f32r, kind="ExternalInput").ap()
        d_b1[s] = nc.dram_tensor(f"b1{s}", [128, 12], f32, kind="ExternalInput").ap()
    d_clsw = nc.dram_tensor("clsw", [128, 8], f32r, kind="ExternalInput").ap()
    d_clsb = nc.dram_tensor("clsb", [1, 1], f32, kind="ExternalInput").ap()
    d_out = nc.dram_tensor("out", [1, BS], f32, kind="ExternalOutput").ap()

    with tile.TileContext(nc) as tc:
        with tc.tile_pool(name="wpool", bufs=1) as wpool, \
             tc.tile_pool(name="persist", bufs=1) as persist:
            # static weights, staged up front on the SWDGE path
            cw_t = wpool.tile([128, DC * U], f32r, name="cw_t")
            nc.gpsimd.dma_start(cw_t[:], d_cw)
            cb_t = wpool.tile([128, UC], f32, name="cb_t")
            nc.gpsimd.dma_start(cb_t[:], d_cb)
            ones_t = wpool.tile([128, 1], bf16, name="ones_t")
            nc.gpsimd.dma_start(ones_t[:], d_ones)
            ones32_t = wpool.tile([1, 4 * GB], f32r, name="ones32_t")
            nc.gpsimd.dma_start(ones32_t[:], d_ones32)
            sewt_t = wpool.tile([W, W], f32r, name="sewt_t")
            nc.gpsimd.dma_start(sewt_t[:], d_sewt)
            seb_t = wpool.tile([1, W], f32r, name="seb_t")
            nc.gpsimd.dma_start(seb_t[:], d_seb)
            w0_t, b0_t, b1_t = {}, {}, {}
            for s in ("f", "r"):
                w0_t[s] = wpool.tile([128, 4 * 1536], f32r, name=f"w0{s}_t")
                nc.gpsimd.dma_start(w0_t[s][:], d_w0[s])
                b0_t[s] = wpool.tile([128, 12], f32, name=f"b0{s}_t")
                nc.gpsimd.dma_start(b0_t[s][:], d_b0[s])
                b1_t[s] = wpool.tile([128, 12], f32, name=f"b1{s}_t")
                nc.gpsimd.dma_start(b1_t[s][:], d_b1[s])
            clsw_t = wpool.tile([128, 8], f32r, name="clsw_t")
            nc.gpsimd.dma_start(clsw_t[:], d_clsw)
            clsb_t = wpool.tile([1, 1], f32, name="clsb_t")
            nc.gpsimd.dma_start(clsb_t[:], d_clsb)

            # pooled^T accumulator [128, uc, BS], filled per group
            pooledT = persist.tile([128, UC, BS], f32r, name="pooledT")

            # l1 forward weights: prefetched during phase 1 (SWDGE path)
            w1_t = {}
            w1_t["f"] = persist.tile([128, 8 * 1536], f32r, name="w1f_t")
            nc.gpsimd.dma_start(w1_t["f"][:], d_w1["f"])

            for _rep in range(unroll):
                # ---------------- phase 1: conv + SE + maxpool ----------------
                with tc.tile_pool(name="xp", bufs=3) as xp, \
                     tc.tile_pool(name="sigp", bufs=10) as sigp, \
                     tc.tile_pool(name="scp", bufs=2) as scp, \
                     tc.tile_pool(name="bcp", bufs=2) as bcp, \
                     tc.tile_pool(name="sep", bufs=2) as sep, \
                     tc.tile_pool(name="drp", bufs=2, space="DRAM") as drp, \
                     tc.tile_pool(name="cps", bufs=6, space="PSUM") as cps, \
                     tc.tile_pool(name="usps", bufs=1, space="PSUM") as usps, \
                     tc.tile_pool(name="lgps", bufs=1, space="PSUM") as lgps:
                    g0 = 0
                    for nblk in BLOCKS:
                        gs = list(range(g0, g0 + nblk))
                        g0 += nblk
                        scr1 = drp.tile([4, GB * W], f32r, name="scr1", tag="scr1")
                        sig_blk = []
                        for gi, g in enumerate(gs):
                            xt = xp.tile([128, DC, GB * W], f32r, name="xt", tag="x")
                            # one 512 KiB DMA per group, alternating HWDGE rings
                            dma_eng = nc.sync if g % 2 == 0 else nc.scalar
                            src = d_xt[:, g * GB:(g + 1) * GB, :].rearrange(
                                "(dc p) b w -> p dc (b w)", p=128
                            )
                            dma_eng.dma_start(xt[:], src)

                            us = usps.tile([1, GB * W], f32, name="us", tag="us")
                            sigg = sigp.tile([128, UC, GB * W], bf16, name="sigg", tag="sig")
                            sig_blk.append(sigg)
                            for uc in range(UC):
                                cp = cps.tile([128, GB * W], f32, name="cp", tag="cp")
                                for dc in range(DC):
                                    nc.tensor.matmul(
                                        cp[:],
                                        cw_t[:, dc * U + uc * 128: dc * U + (uc + 1) * 128],
                                        xt[:, dc, :],
                                        start=(dc == 0),
                                        stop=(dc == DC - 1),
                                    )
                                nc.scalar.activation(
                                    sigg[:, uc, :], cp[:], AF.Sigmoid,
                                    bias=cb_t[:, uc:uc + 1], scale=1.0,
                                )
                                nc.tensor.matmul(
                                    us[:], ones_t[:], sigg[:, uc, :],
                                    start=(uc == 0), stop=(uc == UC - 1),
                                )
                            # avg row -> DRAM scratch (DVE copy keeps ACT on Sigmoid)
                            avg_row = sep.tile([1, GB * W], f32r, name="avg_row", tag="avgrow")
                            nc.vector.tensor_copy(avg_row[:], us[:])
                            nc.sync.dma_start(scr1[gi:gi + 1, :], avg_row[:])

                        # SE for the whole block: avgT [w, (gi b)]
                        nb = len(gs) * GB
                        avgT = sep.tile([W, 4 * GB], f32r, name="avgT", tag="avgT")
                        nc.sync.dma_start(
                            avgT[:, 0:nb],
                            scr1[0:len(gs), :].rearrange("g (b w) -> (w) g b", w=W),
                        )
                        lg = lgps.tile([4 * GB, W], f32, name="lg", tag="lg")
                        nc.tensor.matmul(lg[0:nb, :], avgT[:, 0:nb], sewt_t[:],
                                         start=True, stop=False)
                        nc.tensor.matmul(lg[0:nb, :], ones32_t[:, 0:nb], seb_t[:],
                                         start=False, stop=True)
                        E = sep.tile([4 * GB, W], f32, name="E", tag="E")
                        nc.scalar.activation(E[0:nb, :], lg[0:nb, :], AF.Exp)
                        S = sep.tile([4 * GB, 1], f32, name="S", tag="S")
                        nc.vector.reduce_sum(S[0:nb, :], E[0:nb, :], axis=AX.X)
                        R = sep.tile([4 * GB, 1], f32, name="R", tag="R")
                        nc.vector.reciprocal(R[0:nb, :], S[0:nb, :])
                        seg = sep.tile([4 * GB, W], f32r, name="seg", tag="seg")
                        nc.vector.tensor_scalar_mul(seg[0:nb, :], E[0:nb, :], R[0:nb, 0:1])
                        scr2 = drp.tile([4 * GB, W], f32r, name="scr2", tag="scr2")
                        nc.scalar.dma_start(scr2[0:nb, :], seg[0:nb, :])
                        # broadcast to all partitions with f32r->bf16 cast (SWDGE)
                        sebc = bcp.tile([128, 4 * GB * W], bf16, name="sebc", tag="sebc")
                        nc.gpsimd.dma_start(
                            sebc[:, 0:nb * W],
                            scr2[0:nb, :].bitcast(f32)
                            .rearrange("b w -> (b w)").unsqueeze(0)
                            .broadcast_to([128, nb * W]),
                        )
                        for gi, g in enumerate(gs):
                            scaled = scp.tile([128, UC, GB * W], bf16, name="scaled", tag="scaled")
                            nc.vector.tensor_mul(
                                scaled[:],
                                sig_blk[gi][:],
                                sebc[:, gi * GB * W:(gi + 1) * GB * W]
                                .unsqueeze(1).broadcast_to([128, UC, GB * W]),
                            )
                            pbf = scp.tile([128, UC * GB], bf16, name="pbf", tag="pbf")
                            nc.vector.tensor_reduce(
                                pbf[:],
                                scaled[:].rearrange("p u (b w) -> p (u b) w", w=W),
                                axis=AX.X,
                                op=ALU.max,
                            )
                            nc.vector.tensor_copy(
                                pooledT[:, :, g * GB:(g + 1) * GB],
                                pbf[:].rearrange("p (u b) -> p u b", u=UC),
                            )

                # ---------------- phase 2: LSTM + classifier ----------------
                with tc.tile_pool(name="w1rp", bufs=1) as w1rp, \
                     tc.tile_pool(name="lp", bufs=2) as lp, \
                     tc.tile_pool(name="op", bufs=1) as op, \
                     tc.tile_pool(name="gps", bufs=6, space="PSUM") as gps, \
                     tc.tile_pool(name="clsps", bufs=1, space="PSUM") as clsps:
                    w1_t["r"] = w1rp.tile([128, 8 * 1536], f32r, name="w1r_t")
                    nc.gpsimd.dma_start(w1_t["r"][:], d_w1["r"])

                    def lstm_dir(w_t, b_t, kcs, rhs_tiles, out_tiles, out_tanh):
                        # i/o gates first (Sigmoid run), then g + tanh(c) (Tanh run)
                        gate_sb = {}
                        for gi, func in ((0, AF.Sigmoid), (2, AF.Sigmoid), (1, AF.Tanh)):
                            for q in range(4):
                                m = gi * 4 + q
                                gp = gps.tile([128, BS], f32, name="gp", tag="gp")
                                for kc in range(kcs):
                                    nc.tensor.matmul(
                                        gp[:],
                                        w_t[:, kc * 1536 + m * 128: kc * 1536 + (m + 1) * 128],
                                        rhs_tiles[kc],
                                        start=(kc == 0),
                                        stop=(kc == kcs - 1),
                                    )
                                gs_ = lp.tile([128, BS], f32, name="gs", tag=f"gate{gi}q{q}")
                                nc.scalar.activation(
                                    gs_[:], gp[:], func, bias=b_t[:, m:m + 1], scale=1.0
                                )
                                gate_sb[(gi, q)] = gs_
                        for q in range(4):
                            cpre = lp.tile([128, BS], f32, name="cpre", tag="cpre")
                            nc.vector.tensor_mul(cpre[:], gate_sb[(0, q)][:], gate_sb[(1, q)][:])
                            tcl = lp.tile([128, BS], f32, name="tcl", tag="tcl")
                            nc.scalar.activation(tcl[:], cpre[:], AF.Tanh)
                            if out_tanh:
                                h = lp.tile([128, BS], f32, name="h", tag="h")
                                nc.vector.tensor_mul(h[:], gate_sb[(2, q)][:], tcl[:])
                                nc.scalar.activation(out_tiles[q], h[:], AF.Tanh)
                            else:
                                nc.vector.tensor_mul(out_tiles[q], gate_sb[(2, q)][:], tcl[:])

                    o0T = [op.tile([128, BS], f32r, name=f"o0T{i}")[:] for i in range(8)]
                    o1T = [op.tile([128, BS], f32r, name=f"o1T{i}")[:] for i in range(8)]
                    pooled_rhs = [pooledT[:, kc, :] for kc in range(UC)]
                    lstm_dir(w0_t["f"], b0_t["f"], 4, pooled_rhs, o0T[0:4], False)
                    lstm_dir(w0_t["r"], b0_t["r"], 4, pooled_rhs, o0T[4:8], False)
                    lstm_dir(w1_t["f"], b1_t["f"], 8, o0T, o1T[0:4], True)
                    lstm_dir(w1_t["r"], b1_t["r"], 8, o0T, o1T[4:8], True)

                    clsp = clsps.tile([1, BS], f32, name="clsp")
                    for kc in range(8):
                        nc.tensor.matmul(
                            clsp[:], clsw_t[:, kc:kc + 1], o1T[kc],
                            start=(kc == 0), stop=(kc == 7),
                        )
                    outsb = lp.tile([1, BS], f32, name="outsb", tag="outsb")
                    nc.scalar.activation(
                        outsb[:], clsp[:], AF.Tanh, bias=clsb_t[0:1, 0:1], scale=1.0
                    )
                    nc.sync.dma_start(d_out, outsb[:])

    nc.compile()
    return nc


def _prep_weights(i):
    """Host-side packing of the replicated (non-batch) tensors."""
    import ml_dtypes

    def f32(a):
        return np.ascontiguousarray(a, dtype=np.float32)

    out = {}
    out["cw"] = f32(i["conv_w"].T.reshape(DC, 128, U).transpose(1, 0, 2).reshape(128, DC * U))
    out["cb"] = f32(i["conv_b"].reshape(UC, 128).T)
    out["onescol"] = np.full((128, 1), 1.0 / U, ml_dtypes.bfloat16)
    out["ones32"] = np.ones((1, 4 * GB), np.float32)
    out["sewt"] = f32(i["se_w"].T)
    out["seb"] = f32(i["se_b"].reshape(1, W))
    igo = np.r_[0:512, 1024:2048]  # drop dead forget gate
    for s, tag in (("f", "l0f"), ("r", "l0r")):
        wT = f32(i[f"w_ih_{tag}"]).T[:, igo]                      # [512, 1536]
        out[f"w0{s}"] = f32(wT.reshape(4, 128, 1536).transpose(1, 0, 2).reshape(128, 4 * 1536))
        bs = (f32(i[f"b_ih_{tag}"]) + f32(i[f"b_hh_{tag}"]))[igo]  # [1536]
        out[f"b0{s}"] = f32(bs.reshape(12, 128).T)
    for s, tag in (("f", "l1f"), ("r", "l1r")):
        wT = f32(i[f"w_ih_{tag}"]).T[:, igo]                      # [1024, 1536]
        out[f"w1{s}"] = f32(wT.reshape(8, 128, 1536).transpose(1, 0, 2).reshape(128, 8 * 1536))
        bs = (f32(i[f"b_ih_{tag}"]) + f32(i[f"b_hh_{tag}"]))[igo]
        out[f"b1{s}"] = f32(bs.reshape(12, 128).T)
    out["clsw"] = f32(i["cls_w"].reshape(2 * H)).reshape(8, 128).T.copy()
    out["clsb"] = f32(i["cls_b"]).reshape(1, 1)
    return out


def _get_nc():
    global _STATE
    if _STATE is None:
        _STATE = _build_bass()
    return _STATE


def make_in_maps(**inputs):
    w = _prep_weights(inputs)
    xt = np.ascontiguousarray(
        np.asarray(inputs["x"], dtype=np.float32).transpose(2, 0, 1)
    )  # [D, B, W]
    maps = []
    for c in range(NC):
        m = dict(w)
        m["xt"] = np.ascontiguousarray(xt[:, c * BS:(c + 1) * BS, :])
        maps.append(m)
    return maps


def kernel(**inputs):
    nc = _get_nc()
    maps = make_in_maps(**inputs)
    res = run_bass_kernel_spmd(nc, maps, core_ids=list(range(NC)))
    out = np.empty((B, 1), np.float32)
    for c in range(NC):
        out[c * BS:(c + 1) * BS, 0] = res.results[c]["out"][0]
    return out
